# revision 12
# baseline (speedup 1.0000x reference)
"""Trainium2 Bass kernel: LayerNorm + QKV projection + RoPE (dense transformer).

Full inputs in, full outputs out. Internally shards the 8192 token rows
(b=2 x n=4096) across 8 NeuronCores (data parallel, 1024 tokens/core).

Matmul precision (default "fp8split"): e4m3 hi/lo decomposition computed
so every product lands at natural scale and the three terms accumulate in
a single PSUM group with no combine pass:

    Xh  = q8(xn)            Xl  = q8(xn - Xh)     (residual, unscaled)
    Xhs = q8(xn / 32)
    Wh  = q8(wT)            Wls = q8((wT - Wh) * 32)
    out = Xh@Wh + Xl@Wh + Xhs@Wls        (rel err ~1.3e-3)

All matmuls run in DoubleRow perf mode (2 k-tiles / instruction, 4x fp16
row rate), so the 3x MAC volume still beats one fp16 pass by ~25%.

Per-core pipeline:
  1. DMA x tile [128, 2048]; LayerNorm stats via bn_stats/bn_aggr;
     xn = (x - mu) * rsqrt(var + eps)          (VectorE)
  2. PE-transpose xn 128x128 blocks -> PSUM; produce Xh/Xhs via ScalarE
     Identity-activations (gamma/beta applied as per-partition scale/bias
     in transposed layout), xn-f32 via Pool, Xl via VectorE subtract
  3. QKV matmuls per 1024-col chunk: 24 DoubleRow matmuls per PSUM bank
     (16 k-tile pairs Xh/Xl vs Wh + 8 pairs Xhs vs Wls)
  4. RoPE on q (VectorE) / k (GPSIMD) with host-precomputed cos/sin tables
  5. DMA out contiguous row blocks; host re-assembles [b, h, n, hd]

QKV_MM_DT=float16|bfloat16 selects the old single-pass 2-byte path.
"""

import os
from contextlib import ExitStack

import numpy as np

import concourse.bass as bass
import concourse.tile as tile
from concourse import bacc, mybir
from concourse.bass_utils import run_bass_kernel_spmd
from concourse.masks import make_identity

# Problem shapes (hardcoded per contract)
B, N, DM = 2, 4096, 2048
NCORES = 8
TOK = B * N            # 8192 total token rows
TPC = TOK // NCORES    # 1024 tokens per core
P = 128
MT = TPC // P          # 8 m-tiles per core
KT = DM // P           # 16 k-tiles (contraction)
HEADS, HD = 16, 128
ECW = 1024             # weight-chunk width (half the e range)
NCH = DM // ECW        # 2 chunks
NB = ECW // 512        # matmul n-slices per chunk (PSUM bank = 512 fp32)
HPC = ECW // HD        # heads per chunk = 8
LN_EPS = 1e-5
ROPE_BASE = 10000.0
RSCALE = 32.0          # weight-residual scale (hi/lo split)

F32 = mybir.dt.float32
FP8 = mybir.dt.float8e4
# "fp8split" (default): e4m3 hi/lo DoubleRow path. float16/bfloat16: the
# single-pass 2-byte path.
MODE = os.environ.get("QKV_MM_DT", "fp8split")
SPLIT = MODE == "fp8split"
MM_DT = FP8 if SPLIT else getattr(mybir.dt, MODE)
WKT = 2 * KT if SPLIT else KT   # weight k-tiles in DRAM/SBUF

_CACHE = {}


def _build_nc(body_reps=None):
    if body_reps is None:
        body_reps = int(os.environ.get("QKV_BODY_REPS", "1"))
    nc = bacc.Bacc("TRN2", target_bir_lowering=False, debug=False,
                   enable_asserts=False, num_devices=NCORES)

    x = nc.dram_tensor("x", [TPC, DM], F32, kind="ExternalInput").ap()
    wts = [
        nc.dram_tensor(f"w{n}", [WKT, P, DM], MM_DT,
                       kind="ExternalInput").ap()
        for n in "qkv"
    ]
    gammaT = nc.dram_tensor("gammaT", [P, KT], F32, kind="ExternalInput").ap()
    betaT = nc.dram_tensor("betaT", [P, KT], F32, kind="ExternalInput").ap()
    cosT = nc.dram_tensor("cosT", [P, MT, HD // 2], F32, kind="ExternalInput").ap()
    sinT = nc.dram_tensor("sinT", [P, MT, HD // 2], F32, kind="ExternalInput").ap()
    outs = [
        nc.dram_tensor(f"{n}_out", [TPC, DM], F32, kind="ExternalOutput").ap()
        for n in "qkv"
    ]

    with tile.TileContext(nc) as tc:
        for _rep in range(body_reps):
            with ExitStack() as ctx:
                _kernel_body(ctx, tc, x, wts, gammaT, betaT, cosT, sinT, outs)
    nc.compile()
    return nc


def _kernel_body(ctx, tc, x, wts, gammaT, betaT, cosT, sinT, outs):
    nc = tc.nc

    singles = ctx.enter_context(tc.tile_pool(name="singles", bufs=1))
    xpool = ctx.enter_context(tc.tile_pool(name="xpool", bufs=2))
    stats_pool = ctx.enter_context(tc.tile_pool(name="stats", bufs=4))
    xnt_pool = ctx.enter_context(tc.tile_pool(name="xnt", bufs=1))
    xnf_pool = ctx.enter_context(tc.tile_pool(name="xnf", bufs=3))
    wt_pool = ctx.enter_context(tc.tile_pool(name="wt", bufs=2))
    stage_pool = ctx.enter_context(tc.tile_pool(name="stage", bufs=4))
    rope_pool = ctx.enter_context(tc.tile_pool(name="rope", bufs=3))
    # One shared PSUM pool (8 banks): phase A transposes + phase B accums
    psum = ctx.enter_context(tc.tile_pool(name="psum", bufs=8, space="PSUM"))

    # One-time constants
    identity = singles.tile([P, P], F32)
    make_identity(nc, identity)
    eps_t = singles.tile([P, 1], F32)
    nc.vector.memset(eps_t, LN_EPS)
    gamma_sb = singles.tile([P, KT], F32)
    nc.sync.dma_start(out=gamma_sb, in_=gammaT)
    beta_sb = singles.tile([P, KT], F32)
    nc.sync.dma_start(out=beta_sb, in_=betaT)
    cos_sb = singles.tile([P, MT, HD // 2], F32)
    nc.sync.dma_start(out=cos_sb, in_=cosT)
    sin_sb = singles.tile([P, MT, HD // 2], F32)
    nc.sync.dma_start(out=sin_sb, in_=sinT)

    # Persistent transposed normalized activations: [p=d_inner, k, t].
    # Split mode k-tile blocks: 0..KT-1 = Xh, KT..2KT-1 = Xl, 2KT.. = Xhs.
    XKT = 3 * KT if SPLIT else KT
    xnt = xnt_pool.tile([P, XKT, TPC], MM_DT)

    # ---- Phase A: LayerNorm + transpose (+ quant split), per m-tile ----
    for m in range(MT):
        x_t = xpool.tile([P, DM], F32)
        nc.sync.dma_start(out=x_t, in_=x[m * P:(m + 1) * P, :])

        xg = x_t.rearrange("p (g s) -> p g s", s=512)
        st = stats_pool.tile([P, 4, nc.vector.BN_STATS_DIM], F32)
        for g in range(4):
            nc.vector.bn_stats(out=st[:, g, :], in_=xg[:, g, :])
        mv = stats_pool.tile([P, nc.vector.BN_AGGR_DIM], F32)
        nc.vector.bn_aggr(out=mv, in_=st)

        # rsig = 1/sqrt(var + eps)
        rsig = stats_pool.tile([P, 1], F32)
        nc.scalar.activation(out=rsig, in_=mv[:, 1:2],
                             func=mybir.ActivationFunctionType.Sqrt,
                             bias=eps_t, scale=1.0)
        nc.vector.reciprocal(out=rsig, in_=rsig)

        # xn = (x - mu) * rsig (in place)
        nc.vector.tensor_scalar(out=x_t, in0=x_t,
                                scalar1=mv[:, 0:1], scalar2=rsig,
                                op0=mybir.AluOpType.subtract,
                                op1=mybir.AluOpType.mult)

        # Transpose each 128x128 block; apply gamma/beta + quantize during
        # the PSUM->SBUF copies
        for k in range(KT):
            pt = psum.tile([P, 512], F32, space="PSUM", name="ps")
            nc.tensor.transpose(pt[:, 0:P], x_t[:, k * P:(k + 1) * P],
                                identity)
            ms = slice(m * P, (m + 1) * P)
            if not SPLIT:
                nc.scalar.activation(out=xnt[:, k, ms], in_=pt[:, 0:P],
                                     func=mybir.ActivationFunctionType.Identity,
                                     bias=beta_sb[:, k:k + 1],
                                     scale=gamma_sb[:, k:k + 1])
            else:
                # xnf = gamma*xnT + beta in f32 (ScalarE, the only engine
                # here allowed to read PSUM besides DVE), then quantize:
                # Xh (DVE), Xhs (GPSIMD), Xl = q8(xnf - Xh) (DVE)
                xnf = xnf_pool.tile([P, P], F32)
                nc.scalar.activation(out=xnf, in_=pt[:, 0:P],
                                     func=mybir.ActivationFunctionType.Identity,
                                     bias=beta_sb[:, k:k + 1],
                                     scale=gamma_sb[:, k:k + 1])
                nc.vector.tensor_scalar_mul(xnt[:, k, ms], xnf, 1.0)
                nc.gpsimd.tensor_scalar_mul(xnt[:, 2 * KT + k, ms], xnf,
                                            float(1.0 / RSCALE))
                nc.vector.tensor_sub(xnt[:, KT + k, ms], xnf, xnt[:, k, ms])

    # ---- Phase B: QKV matmuls + RoPE + store ----
    for wi, (w_dram, o_dram) in enumerate(zip(wts, outs)):
        for c in range(NCH):
            w_sb = wt_pool.tile([P, WKT, ECW], MM_DT)
            for k in range(WKT):
                nc.sync.dma_start(out=w_sb[:, k, :],
                                  in_=w_dram[k, :, c * ECW:(c + 1) * ECW])

            for m in range(MT):
                accs = [psum.tile([P, 512], F32, space="PSUM", name="ps")
                        for _ in range(NB)]
                ms = slice(m * P, (m + 1) * P)
                if SPLIT:
                    # (x-block, w-block) pairs, k-tile offsets into xnt/w_sb
                    groups = [(0, 0), (KT, 0), (2 * KT, KT)]
                    for gi, (xo, wo) in enumerate(groups):
                        for k in range(0, KT, 2):
                            lhsT = xnt[:, xo + k:xo + k + 2, ms]
                            for n in range(NB):
                                nc.tensor.matmul(
                                    accs[n], lhsT=lhsT,
                                    rhs=w_sb[:, wo + k:wo + k + 2,
                                             n * 512:(n + 1) * 512],
                                    start=(gi == 0 and k == 0),
                                    stop=(gi == 2 and k == KT - 2),
                                    perf_mode=mybir.MatmulPerfMode.DoubleRow,
                                )
                else:
                    for k in range(KT):
                        lhsT = xnt[:, k, ms]
                        for n in range(NB):
                            nc.tensor.matmul(
                                accs[n], lhsT=lhsT,
                                rhs=w_sb[:, k, n * 512:(n + 1) * 512],
                                start=(k == 0), stop=(k == KT - 1),
                            )

                stg = stage_pool.tile([P, ECW], F32)
                for n in range(NB):
                    nc.scalar.activation(
                        out=stg[:, n * 512:(n + 1) * 512], in_=accs[n],
                        func=mybir.ActivationFunctionType.Copy)

                if wi < 2:  # rope on q and k
                    eng = nc.vector if wi == 0 else nc.gpsimd
                    ov = stg.rearrange("p (h d) -> p h d", d=HD)
                    q1 = ov[:, :, 0:HD // 2]
                    q2 = ov[:, :, HD // 2:HD]
                    cos_m = cos_sb[:, m, :]
                    sin_m = sin_sb[:, m, :]
                    cos_b = bass.AP(tensor=cos_m.tensor, offset=cos_m.offset,
                                    ap=[cos_m.ap[0], [0, HPC], cos_m.ap[1]])
                    sin_b = bass.AP(tensor=sin_m.tensor, offset=sin_m.offset,
                                    ap=[sin_m.ap[0], [0, HPC], sin_m.ap[1]])
                    ta = rope_pool.tile([P, HPC, HD // 2], F32,
                                        name=f"ropeA{wi}")
                    tb = rope_pool.tile([P, HPC, HD // 2], F32,
                                        name=f"ropeB{wi}")
                    eng.tensor_mul(ta, q1, sin_b)      # A = q1*sin
                    eng.tensor_mul(tb, q2, sin_b)      # B = q2*sin
                    eng.tensor_mul(q1, q1, cos_b)      # q1 = q1*cos
                    eng.tensor_sub(q1, q1, tb)         # q1 -= B
                    eng.tensor_mul(q2, q2, cos_b)      # q2 = q2*cos
                    eng.tensor_add(q2, q2, ta)         # q2 += A
                nc.sync.dma_start(
                    out=o_dram[m * P:(m + 1) * P, c * ECW:(c + 1) * ECW],
                    in_=stg)


def _host_prep(x, ln_gamma, ln_beta, wq, wk, wv):
    """Shard/layout inputs. Returns per-core input maps."""
    xf = np.ascontiguousarray(x.reshape(TOK, DM), dtype=np.float32)
    wdt = mybir.dt.np(MM_DT)

    def tile_w(w):
        wt = np.asarray(w, np.float32).T  # [d, e]
        if SPLIT:
            wh = wt.astype(wdt)
            wl = ((wt - wh.astype(np.float32)) * RSCALE).astype(wdt)
            wh = wh.reshape(KT, P, DM)
            wl = wl.reshape(KT, P, DM)
            return np.ascontiguousarray(np.concatenate([wh, wl], axis=0))
        return np.ascontiguousarray(wt.reshape(KT, P, DM)).astype(wdt)

    wq_t, wk_t, wv_t = tile_w(wq), tile_w(wk), tile_w(wv)
    gammaT = np.ascontiguousarray(
        np.asarray(ln_gamma, np.float32).reshape(KT, P).T)
    betaT = np.ascontiguousarray(
        np.asarray(ln_beta, np.float32).reshape(KT, P).T)
    # Build RoPE tables with jax.numpy, matching the reference's fp32 trig
    # bit-for-bit (numpy's fp32 cos differs by ~3e-4 at large arguments).
    import jax.numpy as jnp
    inv_freq = 1.0 / (ROPE_BASE ** (jnp.arange(0, HD, 2, dtype=jnp.float32) / HD))
    t = jnp.arange(N, dtype=jnp.float32)
    freqs = jnp.einsum("i,j->ij", t, inv_freq)  # [N, 64]
    cos_full = np.asarray(jnp.cos(freqs), dtype=np.float32)
    sin_full = np.asarray(jnp.sin(freqs), dtype=np.float32)

    in_maps = []
    for c in range(NCORES):
        pos0 = (c * TPC) % N
        cos_c = np.ascontiguousarray(
            cos_full[pos0:pos0 + TPC].reshape(MT, P, HD // 2).transpose(1, 0, 2))
        sin_c = np.ascontiguousarray(
            sin_full[pos0:pos0 + TPC].reshape(MT, P, HD // 2).transpose(1, 0, 2))
        in_maps.append({
            "x": np.ascontiguousarray(xf[c * TPC:(c + 1) * TPC]),
            "wq": wq_t, "wk": wk_t, "wv": wv_t,
            "gammaT": gammaT, "betaT": betaT,
            "cosT": cos_c, "sinT": sin_c,
        })
    return in_maps


def _assemble(res_list, name):
    full = np.concatenate([res_list[c][name] for c in range(NCORES)], axis=0)
    return np.ascontiguousarray(
        full.reshape(B, N, HEADS, HD).transpose(0, 2, 1, 3))


def kernel(x, ln_gamma, ln_beta, wq, wk, wv, num_heads, _trace=False):
    assert int(num_heads) == HEADS
    in_maps = _host_prep(x, ln_gamma, ln_beta, wq, wk, wv)
    if "nc" not in _CACHE:
        _CACHE["nc"] = _build_nc()
    nc = _CACHE["nc"]
    r = run_bass_kernel_spmd(nc, in_maps, core_ids=list(range(NCORES)),
                             trace=_trace)
    if _trace:
        _CACHE["last_results"] = r
    q = _assemble(r.results, "q_out")
    k = _assemble(r.results, "k_out")
    v = _assemble(r.results, "v_out")
    return q, k, v


# revision 18
# speedup vs baseline: 2.0097x; 2.0097x over previous
"""Trainium2 Bass kernel: LayerNorm + QKV projection + RoPE (dense transformer).

Full inputs in, full outputs out. Internally shards the 8192 token rows
(b=2 x n=4096) across 8 NeuronCores (data parallel, 1024 tokens/core).

Matmul dtype float16 (default): measured fastest on this silicon. fp8
DoubleRow paths (QKV_MM_DT=fp8split etc.) were measured slower — DoubleRow
disables fast-weight-load and its 256-column LDWEIGHTS is exposed, so the
2x MAC rate never materializes (1term 331us vs fp16 289us), and the
accuracy-preserving 3-term hi/lo split costs 3x volume (742us).

Per-core pipeline (phase A interleaved with the first weight chunk):
  1. DMA x tile [128, 2048]; LayerNorm stats via bn_stats/bn_aggr;
     xh = fp16((x - mu) * rsqrt(var + eps))    (VectorE, fp16 out)
  2. PE-transpose xh 128x128 blocks (fp16: 1 cycle/row) -> PSUM;
     ScalarE Identity-copy to SBUF applying gamma/beta (per-partition
     scale/bias in transposed layout)
  3. Right after each m-tile of phase A, the (q, chunk0) matmuls for that
     m-tile run -- PE never idles waiting for LayerNorm. Remaining 5
     weight chunks follow, double-buffered.
  4. QKV matmuls: out[t, e] accumulated over 16 k-tiles in PSUM
  5. RoPE on q (VectorE) / k (GPSIMD) with host-precomputed cos/sin tables
  6. DMA out contiguous row blocks; host re-assembles [b, h, n, hd]
"""

import os
from contextlib import ExitStack

import numpy as np

import concourse.bass as bass
import concourse.tile as tile
from concourse import bacc, mybir
from concourse.bass_utils import run_bass_kernel_spmd
from concourse.masks import make_identity

# Problem shapes (hardcoded per contract)
B, N, DM = 2, 4096, 2048
NCORES = 8
TOK = B * N            # 8192 total token rows
TPC = TOK // NCORES    # 1024 tokens per core
P = 128
MT = TPC // P          # 8 m-tiles per core
KT = DM // P           # 16 k-tiles (contraction)
HEADS, HD = 16, 128
ECW = 1024             # weight-chunk width (half the e range)
NCH = DM // ECW        # 2 chunks
NB = ECW // 512        # matmul n-slices per chunk (PSUM bank = 512 fp32)
HPC = ECW // HD        # heads per chunk = 8
LN_EPS = 1e-5
ROPE_BASE = 10000.0

F32 = mybir.dt.float32
MODE = os.environ.get("QKV_MM_DT", "float16")
MM_DT = getattr(mybir.dt, MODE)

_CACHE = {}


def _build_nc(body_reps=None):
    if body_reps is None:
        body_reps = int(os.environ.get("QKV_BODY_REPS", "1"))
    nc = bacc.Bacc("TRN2", target_bir_lowering=False, debug=False,
                   enable_asserts=False, num_devices=NCORES)

    x = nc.dram_tensor("x", [TPC, DM], F32, kind="ExternalInput").ap()
    wts = [
        nc.dram_tensor(f"w{n}", [KT, P, DM], MM_DT,
                       kind="ExternalInput").ap()
        for n in "qkv"
    ]
    gammaT = nc.dram_tensor("gammaT", [P, KT], F32, kind="ExternalInput").ap()
    betaT = nc.dram_tensor("betaT", [P, KT], F32, kind="ExternalInput").ap()
    cosT = nc.dram_tensor("cosT", [P, MT, HD // 2], F32, kind="ExternalInput").ap()
    sinT = nc.dram_tensor("sinT", [P, MT, HD // 2], F32, kind="ExternalInput").ap()
    outs = [
        nc.dram_tensor(f"{n}_out", [TPC, DM], F32, kind="ExternalOutput").ap()
        for n in "qkv"
    ]

    with tile.TileContext(nc) as tc:
        for _rep in range(body_reps):
            with ExitStack() as ctx:
                _kernel_body(ctx, tc, x, wts, gammaT, betaT, cosT, sinT, outs)
    nc.compile()
    return nc


def _kernel_body(ctx, tc, x, wts, gammaT, betaT, cosT, sinT, outs):
    nc = tc.nc

    singles = ctx.enter_context(tc.tile_pool(name="singles", bufs=1))
    xpool = ctx.enter_context(tc.tile_pool(name="xpool", bufs=2))
    xhpool = ctx.enter_context(tc.tile_pool(name="xhpool", bufs=2))
    stats_pool = ctx.enter_context(tc.tile_pool(name="stats", bufs=4))
    xnt_pool = ctx.enter_context(tc.tile_pool(name="xnt", bufs=1))
    wt_pool = ctx.enter_context(tc.tile_pool(name="wt", bufs=2))
    stage_pool = ctx.enter_context(tc.tile_pool(name="stage", bufs=4))
    rope_pool = ctx.enter_context(tc.tile_pool(name="rope", bufs=3))
    # One shared PSUM pool (8 banks): phase A transposes + phase B accums
    psum = ctx.enter_context(tc.tile_pool(name="psum", bufs=8, space="PSUM"))

    # One-time constants
    identity = singles.tile([P, P], MM_DT)
    make_identity(nc, identity)
    eps_t = singles.tile([P, 1], F32)
    nc.vector.memset(eps_t, LN_EPS)
    gamma_sb = singles.tile([P, KT], F32)
    nc.sync.dma_start(out=gamma_sb, in_=gammaT)
    beta_sb = singles.tile([P, KT], F32)
    nc.sync.dma_start(out=beta_sb, in_=betaT)
    cos_sb = singles.tile([P, MT, HD // 2], F32)
    nc.sync.dma_start(out=cos_sb, in_=cosT)
    sin_sb = singles.tile([P, MT, HD // 2], F32)
    nc.sync.dma_start(out=sin_sb, in_=sinT)

    # Persistent transposed normalized activations: [p=d_inner, k, t]
    xnt = xnt_pool.tile([P, KT, TPC], MM_DT)

    def load_chunk(wi, c):
        w_sb = wt_pool.tile([P, KT, ECW], MM_DT)
        for k in range(KT):
            nc.sync.dma_start(out=w_sb[:, k, :],
                              in_=wts[wi][k, :, c * ECW:(c + 1) * ECW])
        return w_sb

    def b_mtile(wi, c, w_sb, m):
        """Matmuls + rope + store for one (projection, chunk, m-tile)."""
        o_dram = outs[wi]
        accs = [psum.tile([P, 512], F32, space="PSUM", name="ps")
                for _ in range(NB)]
        ms = slice(m * P, (m + 1) * P)
        for k in range(KT):
            lhsT = xnt[:, k, ms]
            for n in range(NB):
                nc.tensor.matmul(
                    accs[n], lhsT=lhsT,
                    rhs=w_sb[:, k, n * 512:(n + 1) * 512],
                    start=(k == 0), stop=(k == KT - 1),
                )

        stg = stage_pool.tile([P, ECW], F32)
        for n in range(NB):
            nc.scalar.activation(
                out=stg[:, n * 512:(n + 1) * 512], in_=accs[n],
                func=mybir.ActivationFunctionType.Copy)

        if wi < 2:  # rope on q and k
            eng = nc.vector if wi == 0 else nc.gpsimd
            ov = stg.rearrange("p (h d) -> p h d", d=HD)
            q1 = ov[:, :, 0:HD // 2]
            q2 = ov[:, :, HD // 2:HD]
            cos_m = cos_sb[:, m, :]
            sin_m = sin_sb[:, m, :]
            cos_b = bass.AP(tensor=cos_m.tensor, offset=cos_m.offset,
                            ap=[cos_m.ap[0], [0, HPC], cos_m.ap[1]])
            sin_b = bass.AP(tensor=sin_m.tensor, offset=sin_m.offset,
                            ap=[sin_m.ap[0], [0, HPC], sin_m.ap[1]])
            ta = rope_pool.tile([P, HPC, HD // 2], F32, name=f"ropeA{wi}")
            tb = rope_pool.tile([P, HPC, HD // 2], F32, name=f"ropeB{wi}")
            eng.tensor_mul(ta, q1, sin_b)      # A = q1*sin
            eng.tensor_mul(tb, q2, sin_b)      # B = q2*sin
            eng.tensor_mul(q1, q1, cos_b)      # q1 = q1*cos
            eng.tensor_sub(q1, q1, tb)         # q1 -= B
            eng.tensor_mul(q2, q2, cos_b)      # q2 = q2*cos
            eng.tensor_add(q2, q2, ta)         # q2 += A
        nc.sync.dma_start(
            out=o_dram[m * P:(m + 1) * P, c * ECW:(c + 1) * ECW],
            in_=stg)

    # First weight chunk (q, c=0) prefetched before phase A
    w_first = load_chunk(0, 0)

    # ---- Phase A (+ interleaved (q, c0) matmuls), per m-tile ----
    for m in range(MT):
        x_t = xpool.tile([P, DM], F32)
        nc.sync.dma_start(out=x_t, in_=x[m * P:(m + 1) * P, :])

        xg = x_t.rearrange("p (g s) -> p g s", s=512)
        st = stats_pool.tile([P, 4, nc.vector.BN_STATS_DIM], F32)
        for g in range(4):
            nc.vector.bn_stats(out=st[:, g, :], in_=xg[:, g, :])
        mv = stats_pool.tile([P, nc.vector.BN_AGGR_DIM], F32)
        nc.vector.bn_aggr(out=mv, in_=st)

        # rsig = 1/sqrt(var + eps)
        rsig = stats_pool.tile([P, 1], F32)
        nc.scalar.activation(out=rsig, in_=mv[:, 1:2],
                             func=mybir.ActivationFunctionType.Sqrt,
                             bias=eps_t, scale=1.0)
        nc.vector.reciprocal(out=rsig, in_=rsig)

        # xh = fp16((x - mu) * rsig)
        xh = xhpool.tile([P, DM], MM_DT)
        nc.vector.tensor_scalar(out=xh, in0=x_t,
                                scalar1=mv[:, 0:1], scalar2=rsig,
                                op0=mybir.AluOpType.subtract,
                                op1=mybir.AluOpType.mult)

        # Transpose each 128x128 block (fp16: 1 cycle/row); apply
        # gamma/beta during the PSUM->SBUF copy
        for k in range(KT):
            pt = psum.tile([P, 512], MM_DT, space="PSUM", name="ps")
            nc.tensor.transpose(pt[:, 0:P], xh[:, k * P:(k + 1) * P],
                                identity)
            nc.scalar.activation(out=xnt[:, k, m * P:(m + 1) * P],
                                 in_=pt[:, 0:P],
                                 func=mybir.ActivationFunctionType.Identity,
                                 bias=beta_sb[:, k:k + 1],
                                 scale=gamma_sb[:, k:k + 1])

        b_mtile(0, 0, w_first, m)

    # ---- Remaining 5 weight chunks ----
    for wi, c in [(0, 1), (1, 0), (1, 1), (2, 0), (2, 1)]:
        w_sb = load_chunk(wi, c)
        for m in range(MT):
            b_mtile(wi, c, w_sb, m)


def _host_prep(x, ln_gamma, ln_beta, wq, wk, wv):
    """Shard/layout inputs. Returns per-core input maps."""
    xf = np.ascontiguousarray(x.reshape(TOK, DM), dtype=np.float32)
    wdt = mybir.dt.np(MM_DT)

    def tile_w(w):
        wt = np.asarray(w, np.float32).T  # [d, e]
        return np.ascontiguousarray(wt.reshape(KT, P, DM)).astype(wdt)

    wq_t, wk_t, wv_t = tile_w(wq), tile_w(wk), tile_w(wv)
    gammaT = np.ascontiguousarray(
        np.asarray(ln_gamma, np.float32).reshape(KT, P).T)
    betaT = np.ascontiguousarray(
        np.asarray(ln_beta, np.float32).reshape(KT, P).T)

    # Build RoPE tables with jax.numpy, matching the reference's fp32 trig
    # bit-for-bit (numpy's fp32 cos differs by ~3e-4 at large arguments).
    import jax.numpy as jnp
    inv_freq = 1.0 / (ROPE_BASE ** (jnp.arange(0, HD, 2, dtype=jnp.float32) / HD))
    t = jnp.arange(N, dtype=jnp.float32)
    freqs = jnp.einsum("i,j->ij", t, inv_freq)  # [N, 64]
    cos_full = np.asarray(jnp.cos(freqs), dtype=np.float32)
    sin_full = np.asarray(jnp.sin(freqs), dtype=np.float32)

    in_maps = []
    for c in range(NCORES):
        pos0 = (c * TPC) % N
        cos_c = np.ascontiguousarray(
            cos_full[pos0:pos0 + TPC].reshape(MT, P, HD // 2).transpose(1, 0, 2))
        sin_c = np.ascontiguousarray(
            sin_full[pos0:pos0 + TPC].reshape(MT, P, HD // 2).transpose(1, 0, 2))
        in_maps.append({
            "x": np.ascontiguousarray(xf[c * TPC:(c + 1) * TPC]),
            "wq": wq_t, "wk": wk_t, "wv": wv_t,
            "gammaT": gammaT, "betaT": betaT,
            "cosT": cos_c, "sinT": sin_c,
        })
    return in_maps


def _assemble(res_list, name):
    full = np.concatenate([res_list[c][name] for c in range(NCORES)], axis=0)
    return np.ascontiguousarray(
        full.reshape(B, N, HEADS, HD).transpose(0, 2, 1, 3))


def kernel(x, ln_gamma, ln_beta, wq, wk, wv, num_heads, _trace=False):
    assert int(num_heads) == HEADS
    in_maps = _host_prep(x, ln_gamma, ln_beta, wq, wk, wv)
    if "nc" not in _CACHE:
        _CACHE["nc"] = _build_nc()
    nc = _CACHE["nc"]
    r = run_bass_kernel_spmd(nc, in_maps, core_ids=list(range(NCORES)),
                             trace=_trace)
    if _trace:
        _CACHE["last_results"] = r
    q = _assemble(r.results, "q_out")
    k = _assemble(r.results, "k_out")
    v = _assemble(r.results, "v_out")
    return q, k, v
